# revision 17
# baseline (speedup 1.0000x reference)
"""Trainium2 Bass kernel for nn_NeuralTuringMachine_59176059404761.

Math (with the reference's degenerate structure exploited):
  gates = x @ W_ih.T (+ b_ih + b_hh);  only i,g,o used (f is dead: c0=0)
  h     = sigmoid(o) * tanh(sigmoid(i) * tanh(g))          [B, 1024]
  read head: softmax over H=1 axis == 1  ->  erase = memory[0] broadcast
  add   = (h @ W_write.T + b_write) * x                    [B, 4096]
  s0,s1 = h @ W_shift.T + b_shift                          [B] each
  out[b] = roll(mem, -trunc(s0[b])) with element (-trunc(s1[b]))%4096 zeroed
           + memory[0]*(1-s1[b]) + add[b]*s1[b]

Sharding: data-parallel over batch across 8 cores (1024 rows each), weights
replicated.  The host pre-packs operands into the SBUF tile layouts the PE
wants (transposed, fp16 hi/lo split) — pure layout marshalling:
  - gates are computed transposed ([g, b]) as a 3-term fp16 hi/lo matmul
    (xh@Wh + xh@Wl + xl@Wh).  This gives fp32-grade gate accuracy; TRN2's
    fp32r is only ~13-bit (measured 1.5e-4) and the trunc() on s0/s1 makes
    gate precision critical (1 ulp can flip a whole output row).
  - sigmoid/tanh: ACT tables (measured 9e-7 / 2.4e-7).
  - the write head runs in fp16 (measured output impact ~3e-4 of scale).
  - the per-sample roll is one aligned row gather (dma_gather, elem_step=64)
    from a DRAM table mem64[r, t] = mem3[r + t]; row index = (s%64)*128+s//64
    where s = trunc(s0) mod 4096.  Gather indices are int16, [16,8]-wrapped
    and replicated to all 8 Q7 core groups (hardware requirement).
  - per-sample zeroing via an int16-iota != zidx mask.
  - batch is processed in two 512-row halves to bound SBUF (W streamed 2x).
"""
import os
import numpy as np

import concourse.bacc as bacc
import concourse.bass as bass
import concourse.mybir as mybir
import concourse.tile as tile
from concourse.bass_utils import run_bass_kernel_spmd
from concourse.masks import make_identity

F32 = mybir.dt.float32
BF16 = mybir.dt.bfloat16
F16 = mybir.dt.float16
I16 = mybir.dt.int16
I32 = mybir.dt.int32
AF = mybir.ActivationFunctionType
ALU = mybir.AluOpType

NCORES = 8
B = 8192
BSH = B // NCORES          # 1024 rows per core
BH = 512                   # rows per half-pass
IN = 4096
HID = 4096
CH = 1024
NB = BSH // 128            # 8 b-tiles per core
NBH = BH // 128            # 4 b-tiles per half
NK = IN // 128             # 32 contraction tiles
NCH = CH // 128            # 8 ch tiles

LAST_RESULTS = None        # test.py reads this

_cache = {}


def _pack_w(Wr):
    """[R, IN] f32 -> hi/lo fp16 slab images [R//128, 128, IN].

    slab s holds lhsT blocks for output rows s*128..s*128+127:
    slab[s][p, k*128 + g] = W[s*128 + g, k*128 + p].
    """
    R = Wr.shape[0]
    wh = Wr.astype(np.float16)
    wl = (Wr - wh.astype(np.float32)).astype(np.float16)
    out = []
    for w in (wh, wl):
        a = w.reshape(R // 128, 128, NK, 128)          # [s, g, k, p]
        a = np.ascontiguousarray(a.transpose(0, 3, 2, 1))  # [s, p, k, g]
        out.append(a.reshape(R // 128, 128, IN))
    return out


def _pack_x(x_sh):
    """[BSH, IN] f32 -> hi/lo fp16 images [2, 128, NK*BH] (per half).

    image[half][p, k*BH + b] = x[half*BH + b, k*128 + p]
    """
    xh = x_sh.astype(np.float16)
    xl = (x_sh - xh.astype(np.float32)).astype(np.float16)
    out = []
    for v in (xh, xl):
        a = v.reshape(2, BH, NK, 128)                  # [half, b, k, p]
        a = np.ascontiguousarray(a.transpose(0, 3, 2, 1))  # [half, p, k, b]
        out.append(a.reshape(2, 128, NK * BH))
    return out


def kernel(**inputs):
    x = np.ascontiguousarray(np.asarray(inputs["x"], dtype=np.float32))
    W_ih = np.asarray(inputs["W_ih"], dtype=np.float32)
    b_ih = np.asarray(inputs["b_ih"], dtype=np.float32)
    b_hh = np.asarray(inputs["b_hh"], dtype=np.float32)
    memory = np.ascontiguousarray(np.asarray(inputs["memory"], dtype=np.float32))
    W_write = np.asarray(inputs["W_write"], dtype=np.float32)
    b_write = np.asarray(inputs["b_write"], dtype=np.float32)
    W_shift = np.ascontiguousarray(np.asarray(inputs["W_shift"], dtype=np.float32))
    b_shift = np.ascontiguousarray(np.asarray(inputs["b_shift"], dtype=np.float32))

    W_igo = np.concatenate([W_ih[0:CH], W_ih[2 * CH:4 * CH]], axis=0)  # [3072, IN]
    bsum = b_ih + b_hh
    bsum_igo = np.ascontiguousarray(
        np.concatenate([bsum[0:CH], bsum[2 * CH:4 * CH]]))             # [3072]
    use_bw = bool(np.any(b_write != 0.0))

    wth, wtl = _pack_w(W_igo)                          # [24, 128, IN] f16 x2
    # W_write.T in fp16, k-tile-major: wwt[k][p, hid] = W_write[hid, k*128+p]
    wwt = np.ascontiguousarray(
        W_write.T.astype(np.float16).reshape(NCH, 128, HID))

    nc = _build(use_bw)

    in_maps = []
    for c in range(NCORES):
        x_sh = x[c * BSH:(c + 1) * BSH]
        xth, xtl = _pack_x(x_sh)
        in_maps.append({
            "x": x_sh,
            "xth": xth, "xtl": xtl,
            "wth": wth, "wtl": wtl,
            "wwt": wwt,
            "bsum_igo": bsum_igo,
            "memory": memory,
            "b_write": b_write.astype(np.float32),
            "W_shift": W_shift,
            "b_shift": b_shift,
        })

    trace = os.environ.get("NTM_TRACE", "0") == "1"
    res = run_bass_kernel_spmd(nc, in_maps, list(range(NCORES)), trace=trace)
    global LAST_RESULTS
    LAST_RESULTS = res
    out = np.concatenate([res.results[c]["out"] for c in range(NCORES)], axis=0)
    return out.astype(np.float32)


def _build(use_bw):
    key = (use_bw,)
    if key in _cache:
        return _cache[key]
    nc = bacc.Bacc("TRN2", target_bir_lowering=False, debug=False)

    d = {
        "x": nc.dram_tensor("x", [BSH, IN], F32, kind="ExternalInput").ap(),
        "xth": nc.dram_tensor("xth", [2, 128, NK * BH], F16, kind="ExternalInput").ap(),
        "xtl": nc.dram_tensor("xtl", [2, 128, NK * BH], F16, kind="ExternalInput").ap(),
        "wth": nc.dram_tensor("wth", [24, 128, IN], F16, kind="ExternalInput").ap(),
        "wtl": nc.dram_tensor("wtl", [24, 128, IN], F16, kind="ExternalInput").ap(),
        "wwt": nc.dram_tensor("wwt", [NCH, 128, HID], F16, kind="ExternalInput").ap(),
        "bsum": nc.dram_tensor("bsum_igo", [3 * CH], F32, kind="ExternalInput").ap(),
        "mem": nc.dram_tensor("memory", [1, HID], F32, kind="ExternalInput").ap(),
        "bw": nc.dram_tensor("b_write", [HID], F32, kind="ExternalInput").ap(),
        "ws": nc.dram_tensor("W_shift", [2, CH], F32, kind="ExternalInput").ap(),
        "bs": nc.dram_tensor("b_shift", [2], F32, kind="ExternalInput").ap(),
        "out": nc.dram_tensor("out", [BSH, HID], F32, kind="ExternalOutput").ap(),
        "mem3": nc.dram_tensor("mem3", [2 * HID + 64], F32).ap(),
        "mem64": nc.dram_tensor("mem64", [64 * 8192], F32).ap(),
        "idx": nc.dram_tensor("idx_scratch", [BSH], I16).ap(),
    }

    with tile.TileContext(nc) as tc:
        _kernel_body(tc, nc, d, use_bw)
    nc.compile()
    _cache[key] = nc
    return nc


def _kernel_body(tc, nc, d, use_bw):
    with tc.tile_pool(name="const", bufs=1) as const:
        ident32 = const.tile([128, 128], F32, tag="id32")
        make_identity(nc, ident32[:])

        # gate biases (b_ih + b_hh): column (sec*8 + ch) = bsum[(sec*8+ch)*128:][:128]
        bsum_t = const.tile([128, 3 * NCH], F32, tag="bsum")
        nc.sync.dma_start(out=bsum_t[:],
                          in_=bass.AP(d["bsum"].tensor, 0, [[1, 128], [128, 3 * NCH]]))

        bsh = const.tile([128, 2], F32, tag="bsh")
        nc.sync.dma_start(out=bsh[:], in_=bass.AP(d["bs"].tensor, 0, [[0, 128], [1, 2]]))

        # W_shift.T packed [128, 16]: ch-tile k at columns 2k
        wsT = const.tile([128, 2 * NCH], F32, tag="wsT")
        ws_sb = const.tile([2, CH], F32, tag="ws")
        nc.sync.dma_start(out=ws_sb[:], in_=d["ws"])
        with tc.tile_pool(name="ps_ws", bufs=1, space="PSUM") as ps_ws:
            ps_w = ps_ws.tile([128, 2 * NCH], F32, tag="psw")
            for k in range(NCH):
                nc.tensor.transpose(ps_w[:, 2 * k:2 * k + 2],
                                    ws_sb[:, k * 128:(k + 1) * 128],
                                    ident32[0:2, 0:2])
            nc.vector.tensor_copy(wsT[:], ps_w[:])

        # roll tables in DRAM: mem3 = [mem, mem, mem[:64]]; mem64[r] = mem3[r:r+8192]
        nc.sync.dma_start(out=d["mem3"][0:HID], in_=d["mem"][0, :])
        nc.sync.dma_start(out=d["mem3"][HID:2 * HID], in_=d["mem"][0, :])
        nc.sync.dma_start(out=d["mem3"][2 * HID:2 * HID + 64], in_=d["mem"][0, 0:64])
        nc.sync.dma_start(out=d["mem64"],
                          in_=bass.AP(d["mem3"].tensor, 0, [[1, 64], [1, 8192]]))

        # persistent: h in f16 [128 ch, 1024 b] x 8, per-sample scalars
        hT = [const.tile([128, BSH], F16, tag=f"hT{k}", name=f"hT{k}")
              for k in range(NCH)]
        sS_t = const.tile([128, 2 * NB], F32, tag="sS")
        s1_t = const.tile([128, NB], F32, tag="s1")
        s1c_t = const.tile([128, NB], F32, tag="s1c")
        zidx_t = const.tile([128, NB], F32, tag="zidx")
        gidx_t = const.tile([128, NB], F32, tag="gidx")
        idxw = const.tile([128, NB * 8], I16, tag="idxw")

        # ================= phase B (per half): gates, h, shift =============
        with (
            tc.tile_pool(name="xT", bufs=1) as xTp,
            tc.tile_pool(name="ld", bufs=3) as ld,
            tc.tile_pool(name="act", bufs=2) as actp,
            tc.tile_pool(name="psG", bufs=4, space="PSUM") as psG,
            tc.tile_pool(name="psS", bufs=2, space="PSUM") as psS,
        ):
            xTh = xTp.tile([128, NK * BH], F16, tag="xTh")
            xTl = xTp.tile([128, NK * BH], F16, tag="xTl")

            for half in range(2):
                nc.sync.dma_start(out=xTh[:], in_=d["xth"][half])
                nc.sync.dma_start(out=xTl[:], in_=d["xtl"][half])

                for ch in range(NCH):
                    res3 = []
                    for sec in range(3):  # 0=i, 1=g, 2=o
                        s = sec * NCH + ch
                        wh = ld.tile([128, IN], F16, tag="wh")
                        wl = ld.tile([128, IN], F16, tag="wl")
                        nc.sync.dma_start(out=wh[:], in_=d["wth"][s])
                        nc.sync.dma_start(out=wl[:], in_=d["wtl"][s])

                        pg = psG.tile([128, BH], F32, tag="ps_g")
                        for k in range(NK):
                            ksl = slice(k * 128, (k + 1) * 128)
                            xsl = slice(k * BH, (k + 1) * BH)
                            nc.tensor.matmul(pg[:], wh[:, ksl], xTh[:, xsl],
                                             start=(k == 0), stop=False)
                            nc.tensor.matmul(pg[:], wh[:, ksl], xTl[:, xsl],
                                             start=False, stop=False)
                            nc.tensor.matmul(pg[:], wl[:, ksl], xTh[:, xsl],
                                             start=False, stop=(k == NK - 1))

                        res = actp.tile([128, BH], F32, tag=f"act{sec}")
                        func = AF.Tanh if sec == 1 else AF.Sigmoid
                        nc.scalar.activation(res[:], pg[:], func,
                                             bias=bsum_t[:, s:s + 1], scale=1.0)
                        res3.append(res)

                    sig_i, tan_g, sig_o = res3
                    # c = sig_i*tan_g (into tan_g); tanh(c) (into sig_i);
                    # h = sig_o*tanh_c (into sig_o)
                    nc.vector.tensor_mul(tan_g[:], sig_i[:], tan_g[:])
                    nc.scalar.activation(sig_i[:], tan_g[:], AF.Tanh)
                    nc.vector.tensor_mul(sig_o[:], sig_o[:], sig_i[:])
                    nc.vector.tensor_copy(hT[ch][:, half * BH:(half + 1) * BH],
                                          sig_o[:])

                    # shift head: per-ch partials (own accumulation groups),
                    # summed in SBUF
                    psS_t = psS.tile([128, 2 * NBH], F32, tag="ps_s")
                    for ml in range(NBH):
                        nc.tensor.matmul(
                            psS_t[:, 2 * ml:2 * ml + 2],
                            sig_o[:, ml * 128:(ml + 1) * 128],
                            wsT[:, 2 * ch:2 * ch + 2],
                            start=True, stop=True)
                    half_sl = slice(2 * half * NBH, 2 * (half + 1) * NBH)
                    if ch == 0:
                        nc.vector.tensor_copy(sS_t[:, half_sl], psS_t[:])
                    else:
                        nc.vector.tensor_tensor(sS_t[:, half_sl], sS_t[:, half_sl],
                                                psS_t[:], op=ALU.add)

            # ---- shift scalars -> roll/zero indices (all fp32-exact) ----
            with tc.tile_pool(name="tiny", bufs=1) as tiny:
                def floor_of(x_ap, tag):
                    xi = tiny.tile([128, 1], I32, tag=f"{tag}_i", name=f"{tag}_i")
                    xf = tiny.tile([128, 1], F32, tag=f"{tag}_f", name=f"{tag}_f")
                    nc.vector.tensor_copy(xi[:], x_ap)
                    nc.vector.tensor_copy(xf[:], xi[:])
                    gt = tiny.tile([128, 1], F32, tag=f"{tag}_gt", name=f"{tag}_gt")
                    nc.vector.tensor_tensor(gt[:], xf[:], x_ap, op=ALU.is_gt)
                    t = tiny.tile([128, 1], F32, tag=f"{tag}_t", name=f"{tag}_t")
                    nc.vector.tensor_sub(t[:], xf[:], gt[:])
                    return t

                def trunc_of(x_ap, tag):
                    xi = tiny.tile([128, 1], I32, tag=f"{tag}_i", name=f"{tag}_i")
                    xf = tiny.tile([128, 1], F32, tag=f"{tag}_f", name=f"{tag}_f")
                    nc.vector.tensor_copy(xi[:], x_ap)
                    nc.vector.tensor_copy(xf[:], xi[:])
                    gt = tiny.tile([128, 1], F32, tag=f"{tag}_gt", name=f"{tag}_gt")
                    lt = tiny.tile([128, 1], F32, tag=f"{tag}_lt", name=f"{tag}_lt")
                    nc.vector.tensor_tensor(gt[:], xf[:], x_ap, op=ALU.is_gt)
                    nc.vector.tensor_tensor(lt[:], xf[:], x_ap, op=ALU.is_lt)
                    pos = tiny.tile([128, 1], F32, tag=f"{tag}_pos", name=f"{tag}_pos")
                    neg = tiny.tile([128, 1], F32, tag=f"{tag}_neg", name=f"{tag}_neg")
                    nc.vector.tensor_scalar(pos[:], x_ap, 0.0, None, ALU.is_gt)
                    nc.vector.tensor_scalar(neg[:], x_ap, 0.0, None, ALU.is_lt)
                    c1 = tiny.tile([128, 1], F32, tag=f"{tag}_c1", name=f"{tag}_c1")
                    c2 = tiny.tile([128, 1], F32, tag=f"{tag}_c2", name=f"{tag}_c2")
                    nc.vector.tensor_mul(c1[:], gt[:], pos[:])
                    nc.vector.tensor_mul(c2[:], lt[:], neg[:])
                    t = tiny.tile([128, 1], F32, tag=f"{tag}_t", name=f"{tag}_t")
                    nc.vector.tensor_sub(t[:], xf[:], c1[:])
                    nc.vector.tensor_add(t[:], t[:], c2[:])
                    return t

                def mod_pow2(x_ap, div, tag):
                    y = tiny.tile([128, 1], F32, tag=f"{tag}_y", name=f"{tag}_y")
                    nc.vector.tensor_scalar(y[:], x_ap, 1.0 / div, None, ALU.mult)
                    fl = floor_of(y[:], f"{tag}_fl")
                    m = tiny.tile([128, 1], F32, tag=f"{tag}_m", name=f"{tag}_m")
                    nc.vector.tensor_scalar(m[:], fl[:], -float(div), x_ap,
                                            ALU.mult, ALU.add)
                    return m

                for m in range(NB):
                    s0 = tiny.tile([128, 1], F32, tag=f"s0_{m}", name=f"s0_{m}")
                    nc.vector.tensor_tensor(s0[:], sS_t[:, 2 * m:2 * m + 1],
                                            bsh[:, 0:1], op=ALU.add)
                    nc.vector.tensor_tensor(s1_t[:, m:m + 1],
                                            sS_t[:, 2 * m + 1:2 * m + 2],
                                            bsh[:, 1:2], op=ALU.add)
                    tr0 = trunc_of(s0[:], f"tr0_{m}")
                    smod = mod_pow2(tr0[:], HID, f"sm_{m}")
                    rr = mod_pow2(smod[:], 64, f"rr_{m}")
                    cc = tiny.tile([128, 1], F32, tag=f"cc_{m}", name=f"cc_{m}")
                    nc.vector.tensor_sub(cc[:], smod[:], rr[:])
                    nc.vector.tensor_scalar(cc[:], cc[:], 1.0 / 64.0, None, ALU.mult)
                    nc.vector.tensor_scalar(gidx_t[:, m:m + 1], rr[:], 128.0,
                                            cc[:], ALU.mult, ALU.add)
                    tr1 = trunc_of(s1_t[:, m:m + 1], f"tr1_{m}")
                    ntr = tiny.tile([128, 1], F32, tag=f"ntr_{m}", name=f"ntr_{m}")
                    nc.vector.tensor_scalar(ntr[:], tr1[:], -1.0, None, ALU.mult)
                    z = mod_pow2(ntr[:], HID, f"z_{m}")
                    nc.vector.tensor_copy(zidx_t[:, m:m + 1], z[:])
                nc.vector.tensor_scalar(s1c_t[:], s1_t[:], -1.0, 1.0,
                                        ALU.mult, ALU.add)

                # idx -> int16 -> DRAM -> [16,8]-wrap replicated to 8 groups
                gidx16 = tiny.tile([128, NB], I16, tag="gidx16")
                nc.vector.tensor_copy(gidx16[:], gidx_t[:])
                nc.sync.dma_start(
                    out=bass.AP(d["idx"].tensor, 0, [[1, 128], [128, NB]]),
                    in_=gidx16[:])
                for g in range(8):
                    nc.sync.dma_start(
                        out=idxw[16 * g:16 * g + 16, :],
                        in_=bass.AP(d["idx"].tensor, 0,
                                    [[1, 16], [128, NB], [16, 8]]))

        # ============ phase C: write head, roll gather, assembly ============
        with (
            tc.tile_pool(name="big", bufs=1) as big,
            tc.tile_pool(name="roll", bufs=3) as rollp,
            tc.tile_pool(name="chnk", bufs=4) as chnk,
            tc.tile_pool(name="psW", bufs=4, space="PSUM") as psW,
        ):
            membc = big.tile([128, HID], F32, tag="membc")
            nc.sync.dma_start(out=membc[:],
                              in_=bass.AP(d["mem"].tensor, 0, [[0, 128], [1, HID]]))
            iot = big.tile([128, HID], I16, tag="iot")
            nc.gpsimd.iota(iot[:], pattern=[[1, HID]], base=0, channel_multiplier=0)
            if use_bw:
                bwb = big.tile([128, HID], F32, tag="bwb")
                nc.sync.dma_start(out=bwb[:],
                                  in_=bass.AP(d["bw"].tensor, 0, [[0, 128], [1, HID]]))

            wwT = [big.tile([128, HID], F16, tag=f"wwT{k}", name=f"wwT{k}")
                   for k in range(NCH)]
            for k in range(NCH):
                nc.sync.dma_start(out=wwT[k][:], in_=d["wwt"][k])

            src_g = bass.AP(d["mem64"].tensor, 0, [[64, 8128], [1, HID]])
            for m in range(NB):
                rolled = rollp.tile([128, HID], F32, tag="rolled")
                nc.gpsimd.dma_gather(
                    out_ap=rolled[:].rearrange("p (o f) -> p o f", o=1),
                    in_ap=src_g,
                    idxs_ap=idxw[:, m * 8:(m + 1) * 8],
                    num_idxs=128, num_idxs_reg=128,
                    elem_size=HID, elem_step=64,
                )
                for n in range(HID // 512):
                    pw = psW.tile([128, 512], F32, tag="ps_w")
                    for k in range(NCH):
                        nc.tensor.matmul(
                            pw[:], hT[k][:, m * 128:(m + 1) * 128],
                            wwT[k][:, n * 512:(n + 1) * 512],
                            start=(k == 0), stop=(k == NCH - 1))
                    xsl = chnk.tile([128, 512], F32, tag="xsl")
                    nc.sync.dma_start(
                        out=xsl[:],
                        in_=d["x"][m * 128:(m + 1) * 128, n * 512:(n + 1) * 512])
                    js = slice(n * 512, (n + 1) * 512)
                    addc = chnk.tile([128, 512], F32, tag="addc")
                    if use_bw:
                        nc.vector.tensor_tensor(addc[:], pw[:], bwb[:, js],
                                                op=ALU.add)
                        nc.vector.tensor_tensor(addc[:], addc[:], xsl[:],
                                                op=ALU.mult)
                    else:
                        nc.vector.tensor_tensor(addc[:], pw[:], xsl[:],
                                                op=ALU.mult)
                    # out = rolled*(iota != zidx) + memb*(1-s1) + add*s1
                    # (the per-partition-scalar multiplies run on ScalarE)
                    outc = chnk.tile([128, 512], F32, tag="outc")
                    nc.scalar.mul(outc[:], membc[:, js], s1c_t[:, m:m + 1])
                    nc.scalar.mul(addc[:], addc[:], s1_t[:, m:m + 1])
                    nc.vector.tensor_add(outc[:], outc[:], addc[:])
                    maskc = chnk.tile([128, 512], BF16, tag="maskc")
                    nc.vector.tensor_scalar(maskc[:], iot[:, js],
                                            zidx_t[:, m:m + 1], None,
                                            ALU.not_equal)
                    rl = rolled[:, js]
                    nc.vector.tensor_mul(rl, rl, maskc[:])
                    nc.vector.tensor_add(outc[:], outc[:], rl)
                    nc.sync.dma_start(
                        out=d["out"][m * 128:(m + 1) * 128, js], in_=outc[:])
